# revision 1
# baseline (speedup 1.0000x reference)
"""AucLoss on 8 TRN2 NeuronCores (Bass SPMD kernel).

Reference (B=8192, C=2048, GAMA=0.3, UNK=0):
    s = sigmoid(x);  pos_i = s[i, y_i];  valid_i = (y_i != 0)
    neg_j = max_c s[j, c] over c not in {y_j, 0}
    t_j = neg_j + GAMA
    sq_sum = sum_{i valid, j} [t_j > pos_i] * (t_j - pos_i)^2
    loss = sq_sum / (p_count + 1) / (B + 1)

Distribution: data-parallel over the batch. Each core processes a
B/8-row shard of x; one AllGather of [neg | valid*pos | valid*pos^2]
(12 KB/core) makes the global vectors available everywhere; every core
then computes the identical final scalar (no further collective).

Device algorithm, per 128-row block (all work in logit space — sigmoid
is monotone, so the masked row-max commutes with it):
  - mask = (iota == y) * -8192   one 4x-mode DVE tensor_scalar (bf16)
  - xm   = x + mask              gpsimd/DVE tensor_tensor (split 3:1)
  - neg_logit = max over [1:C) of xm    (2x tensor_scalar accum reduce;
    the [1:C) range handles the UNK column)
  - pos_logit = min over [1:C) of xm + 8192: the masked entry
    x[y]-8192 always wins the min. For y == 0 rows the value is junk,
    but those rows are invalid and every downstream term multiplies
    them by valid == 0.
The pairwise term needs no O(B^2) work:
    sum_{i valid, j} (t_j - pos_i)^2 = B*S2 - 2*T1*S1 + T2*P
with T1 = sum t, T2 = sum t^2, S1 = sum v*pos, S2 = sum v*pos^2,
P = sum v. The margin indicator [t_j > pos_i] is enforced by a
runtime-guarded correction: if max(valid*pos) >= min(t) (checked on
device), each core computes sum relu(pos_i - t_j)^2 over all pairs via
ACT Relu + Square(accum) passes and subtracts it. For this problem's
regime (t > 1 > pos always) the guard never fires, so the correction
costs one scalar branch.

Operating range: |x| must be < ~80 (the -8192 mask offset and the
sigmoid/underflow assumptions need |x| tiny relative to 8192; the
reference regime is randn).

Toolchain workarounds for this container's walrus build:
  - any instruction may carry at most ONE sync wait -> extra waits are
    hoisted onto same-engine NOPs after Tile scheduling
  - custom gpsimd ucode (local_scatter etc.) does not codegen -> the
    iota and the transpose identity are tiny host-supplied constants
"""

from contextlib import ExitStack

import numpy as np

import concourse.bass as bass
import concourse.mybir as mybir
import concourse.tile as tile
from concourse.vector_clock import ScopedClock

F32 = mybir.dt.float32
I32 = mybir.dt.int32
ALU = mybir.AluOpType
ACTF = mybir.ActivationFunctionType

B_FULL, C_FULL, N_CORES, GAMA = 8192, 2048, 8, 0.3


class _PatchedTileContext(tile.TileContext):
    """TileContext whose tail drain splits sem waits one per instruction."""

    def _drain_and_barrier(self, tick_clock, wait_clock):
        nc = self.nc
        drain_inst = nc.sync.drain()
        wait_clock.add_sem_waits(
            drain_inst.ins, ScopedClock({None: tick_clock.global_clock})
        )
        si = drain_inst.ins.sync_info
        if si is not None and si.on_wait and len(si.on_wait) > 1:
            extra = list(si.on_wait[1:])
            del si.on_wait[1:]
            for w in extra:
                ni = nc.sync.nop()
                nsi = ni.ins.sync_info
                if nsi is None:
                    ni.ins.sync_info = mybir.SyncInfo(on_wait=[w], on_update=[])
                else:
                    nsi.on_wait.append(w)

        nc.all_engine_barrier()
        assert self.sems is not None
        popped = nc._tile_sem_poison_stack.pop()
        assert popped is self._sem_poison
        nc.clear_and_free_semaphores(list(self.sems.allocated().values()))
        nc.all_engine_barrier()


def _split_multi_waits(nc):
    """This walrus allows one sync wait per instruction; hoist extras onto
    same-engine NOPs inserted immediately before the owning instruction."""
    n = 0
    for f in nc.m.functions:
        for bb in f.blocks:
            out = []
            for ins in bb.instructions:
                si = ins.sync_info
                if si is not None and si.on_wait and len(si.on_wait) > 1:
                    extra = list(si.on_wait[:-1])
                    del si.on_wait[:-1]
                    for w in extra:
                        n += 1
                        out.append(mybir.InstNoOp(
                            name=f"waitnop_{n}",
                            engine=ins.engine,
                            ins=[],
                            outs=[],
                            sync_info=mybir.SyncInfo(on_wait=[w], on_update=[]),
                        ))
                out.append(ins)
            bb.instructions[:] = out
    return n


def _build(B=B_FULL, C=C_FULL, n_cores=N_CORES, gama=GAMA):
    R = B // n_cores
    nb = R // 128
    assert R % 128 == 0
    MASKVAL = -8192.0

    nc = bass.Bass("TRN2", target_bir_lowering=False, debug=False,
                   num_devices=n_cores)
    x_ap = nc.dram_tensor("x", [R, C], F32, kind="ExternalInput").ap()
    y_ap = nc.dram_tensor("yt", [128, nb], I32, kind="ExternalInput").ap()
    iota_ap = nc.dram_tensor("iota2", [128, C],
                             mybir.dt.int16, kind="ExternalInput").ap()
    ident_ap = nc.dram_tensor("ident", [128, 128], F32,
                              kind="ExternalInput").ap()
    out_ap = nc.dram_tensor("out", [1], F32, kind="ExternalOutput").ap()

    CH = 3 * R                # per-core allgather chunk: neg | vpos | vpos2
    groups = [list(range(n_cores))]

    with _PatchedTileContext(nc) as tc:
        with ExitStack() as stk:
            persist = stk.enter_context(tc.tile_pool(name="persist", bufs=1))
            dram = stk.enter_context(
                tc.tile_pool(name="dram", bufs=1, space="DRAM"))
            psum = stk.enter_context(
                tc.tile_pool(name="psum", bufs=1, space="PSUM"))

            ident = persist.tile([128, 128], F32)
            nc.sync.dma_start(out=ident[:], in_=ident_ap)
            iota2 = persist.tile([128, C], mybir.dt.int16)
            nc.sync.dma_start(out=iota2[:], in_=iota_ap)
            ones = persist.tile([128, 1], F32)
            nc.vector.memset(ones[:], 1.0)
            ones2 = persist.tile([2, 1], F32)
            nc.vector.memset(ones2[:], 1.0)

            y32 = persist.tile([128, nb], I32)
            nc.sync.dma_start(out=y32[:], in_=y_ap)
            valid = persist.tile([128, nb], F32)
            nc.vector.tensor_scalar(valid[:], y32[:], 0, None, ALU.not_equal)
            yf = persist.tile([128, nb], F32)
            nc.vector.tensor_copy(yf[:], y32[:])

            negl = persist.tile([128, nb], F32)
            posm = persist.tile([128, nb], F32)

            # trigger the sigmoid ACT table load early so the ~2.7us
            # PSEUDO_LOAD overlaps with phase-1 DMA/compute
            warm = persist.tile([1, 1], F32)
            nc.scalar.activation(warm[:], ones[0:1, 0:1], ACTF.Sigmoid)

            # ---- phase 1: per-block masked rowmax + label-col min ----
            with tc.tile_pool(name="xp", bufs=3) as xp, \
                 tc.tile_pool(name="mp", bufs=3) as mp, \
                 tc.tile_pool(name="dp", bufs=3) as dp:
                for b in range(nb):
                    xb = xp.tile([128, C], F32, tag="x")
                    nc.sync.dma_start(out=xb[:],
                                      in_=x_ap[128 * b:128 * (b + 1), :])

                    mask = mp.tile([128, C], mybir.dt.bfloat16, tag="mask")
                    nc.vector.tensor_scalar(
                        mask[:], iota2[:], yf[:, b:b + 1], MASKVAL,
                        ALU.is_equal, ALU.mult)

                    xm = dp.tile([128, C], F32, tag="dummy")
                    # x + mask: gpsimd (default ucode lib) takes 3 of 4
                    # blocks; DVE takes every 4th to balance engine time
                    eng = nc.vector if b % 4 == 3 else nc.gpsimd
                    eng.tensor_tensor(out=xm[:], in0=xb[:], in1=mask[:],
                                      op=ALU.add)
                    jk1 = dp.tile([128, C - 1], F32, tag="jk1")
                    jk2 = dp.tile([128, C - 1], F32, tag="jk2")
                    # tensor_scalar with accum_out reduces via op1 at 2x
                    # (plain tensor_reduce is 1x)
                    nc.vector.tensor_scalar(
                        jk1[:], xm[:, 1:C], 0.0, None, ALU.add, ALU.max,
                        accum_out=negl[:, b:b + 1])
                    nc.vector.tensor_scalar(
                        jk2[:], xm[:, 1:C], 0.0, None, ALU.add, ALU.min,
                        accum_out=posm[:, b:b + 1])

            # ---- phase 2: sigmoids, packed local stats, one AllGather ----
            pos = persist.tile([128, nb], F32)
            lstats = persist.tile([128, 3, nb], F32)
            neg = lstats[:, 0, :]
            vpos = lstats[:, 1, :]
            vpos2 = lstats[:, 2, :]
            b8k = persist.tile([128, 1], F32)
            nc.vector.memset(b8k[:], -MASKVAL)
            nc.scalar.activation(pos[:], posm[:], ACTF.Sigmoid, bias=b8k[:])
            nc.scalar.activation(neg, negl[:], ACTF.Sigmoid)

            nc.vector.tensor_tensor(out=vpos, in0=pos[:], in1=valid[:],
                                    op=ALU.mult)
            nc.vector.tensor_tensor(out=vpos2, in0=vpos, in1=pos[:],
                                    op=ALU.mult)

            chunk = dram.tile([CH], F32)
            nc.sync.dma_start(
                out=chunk[0:3 * R].rearrange("(s p b) -> p s b", s=3, p=128),
                in_=lstats[:])

            ag = dram.tile([n_cores * CH], F32)
            nc.gpsimd.collective_compute(
                "AllGather", ALU.bypass, replica_groups=groups,
                ins=[chunk.opt()], outs=[ag.opt()])

            # ---- gathered global tiles ----
            nbg = n_cores * nb
            gall = persist.tile([128, 3, nbg], F32)
            negall = gall[:, 0, :]
            vposall = gall[:, 1, :]
            vpos2all = gall[:, 2, :]
            # one DMA per segment: in-AP walks (p)(k, b) with element
            # strides (nb)(3R, 1); out is the contiguous [128, nbg] plane
            for s in range(3):
                nc.sync.dma_start(
                    out=gall[:, s, :].rearrange("p (k b) -> p k b",
                                                k=n_cores),
                    in_=ag[:].rearrange("(k s p b) -> s p k b",
                                        k=n_cores, s=3, p=128)[s])

            tt_ = persist.tile([128, nbg], F32)
            nc.vector.tensor_scalar(tt_[:], negall[:], float(gama), None,
                                    ALU.add)

            # per-partition stats -> ones-matmul partition sum
            stats = persist.tile([128, 5], F32)
            nc.vector.tensor_reduce(stats[:, 0:1], tt_[:],
                                    mybir.AxisListType.X, ALU.add)
            dummyg = persist.tile([128, nbg], F32)
            nc.vector.tensor_tensor(out=dummyg[:], in0=tt_[:], in1=tt_[:],
                                    op=ALU.mult)
            nc.vector.tensor_reduce(stats[:, 1:2], dummyg[:],
                                    mybir.AxisListType.X, ALU.add)
            nc.vector.tensor_reduce(stats[:, 2:3], vposall[:],
                                    mybir.AxisListType.X, ALU.add)
            nc.vector.tensor_reduce(stats[:, 3:4], vpos2all[:],
                                    mybir.AxisListType.X, ALU.add)
            # p_count: valid rows have pos = sigmoid(..) > 0 exactly
            vp01 = persist.tile([128, nbg], F32)
            nc.vector.tensor_scalar(vp01[:], vposall[:], 0.0, None,
                                    ALU.is_gt, ALU.add,
                                    accum_out=stats[:, 4:5])

            pstats = psum.tile([1, 5], F32)
            nc.tensor.matmul(pstats[:], ones[:], stats[:], start=True,
                             stop=True)
            g = persist.tile([1, 5], F32)
            nc.vector.tensor_copy(g[:], pstats[:])
            Pk = g[0:1, 4:5]

            # main = B*S2 - 2*T1*S1 + T2*P, fused:
            m2 = persist.tile([1, 1], F32)
            nc.vector.scalar_tensor_tensor(m2[:], g[0:1, 0:1], -2.0,
                                           g[0:1, 2:3], ALU.mult, ALU.mult)
            m3 = persist.tile([1, 1], F32)
            nc.vector.tensor_tensor(out=m3[:], in0=g[0:1, 1:2], in1=Pk,
                                    op=ALU.mult)
            m13 = persist.tile([1, 1], F32)
            nc.vector.scalar_tensor_tensor(m13[:], g[0:1, 3:4], float(B),
                                           m3[:], ALU.mult, ALU.add)
            main = persist.tile([1, 1], F32)
            nc.vector.tensor_tensor(out=main[:], in0=m13[:], in1=m2[:],
                                    op=ALU.add)
            den = persist.tile([1, 1], F32)
            nc.vector.tensor_scalar(den[:], Pk, 1.0, float(B) + 1.0,
                                    ALU.add, ALU.mult)
            rec = persist.tile([1, 1], F32)
            nc.vector.reciprocal(rec[:], den[:])

            corr = persist.tile([1, 1], F32)
            nc.vector.memset(corr[:], 0.0)

            # guard: max(valid*pos) >= min(t) <=> some pair has t <= pos
            mm2 = persist.tile([128, 2], F32)
            nc.vector.tensor_reduce(mm2[:, 0:1], vposall[:],
                                    mybir.AxisListType.X, ALU.max)
            negt = persist.tile([128, nbg], F32)
            nc.vector.tensor_scalar(negt[:], tt_[:], -1.0, None, ALU.mult)
            nc.vector.tensor_reduce(mm2[:, 1:2], negt[:],
                                    mybir.AxisListType.X, ALU.max)
            # cross-partition max via PE transpose, then a K=2 ones-matmul
            pmt = psum.tile([2, 128], F32, tag="pmt")
            nc.tensor.transpose(pmt[:], mm2[:], ident[:])
            gm = persist.tile([2, 1], F32)
            nc.vector.tensor_reduce(gm[:], pmt[:], mybir.AxisListType.X,
                                    ALU.max)
            pg = psum.tile([1, 1], F32, tag="pg")
            nc.tensor.matmul(pg[:], ones2[:], gm[:], start=True, stop=True)
            flag = persist.tile([1, 1], I32)
            nc.vector.tensor_scalar(flag[:], pg[:], 0.0, None, ALU.is_ge)

            # branch condition must live in a register on every engine
            tmp = nc.alloc_registers(f"corr_flag_{nc.next_id()}",
                                     mybir.ALL_ENGINES)
            nc.regs_load(tmp, flag[0:1, 0:1])
            rv = nc.snap(tmp, donate=True, min_val=0, max_val=1)
            with tc.If(rv == 1):
                # full [B, B] correction, computed redundantly per core:
                # sum over all pairs of relu(pos_i - t_j)^2
                tflat = dram.tile([B], F32)
                nc.sync.dma_start(
                    out=tflat[:].rearrange("(p b) -> p b", p=128),
                    in_=tt_[:])
                tb1 = persist.tile([1, B], F32)
                nc.sync.dma_start(out=tb1[:], in_=tflat[:].rearrange(
                    "(o n) -> o n", o=1))
                # broadcast t to all partitions via K=1 ones-matmuls
                tb = persist.tile([128, B], F32)
                onesb = persist.tile([1, 128], F32)
                nc.vector.memset(onesb[:], 1.0)
                CBC = 512
                for j in range(0, B, CBC):
                    pbc = psum.tile([128, CBC], F32, tag="pbc")
                    nc.tensor.matmul(pbc[:], onesb[:], tb1[0:1, j:j + CBC],
                                     start=True, stop=True)
                    nc.vector.tensor_copy(tb[:, j:j + CBC], pbc[:])
                cacc = persist.tile([128, nbg], F32)
                with tc.tile_pool(name="cp", bufs=1) as cp:
                    for c in range(nbg):
                        r1 = cp.tile([128, B], F32, tag="r1")
                        nc.scalar.activation(r1[:], tb[:], ACTF.Relu,
                                             bias=vposall[:, c:c + 1],
                                             scale=-1.0)
                        r2 = cp.tile([128, B], F32, tag="r2")
                        nc.scalar.activation(r2[:], r1[:], ACTF.Square,
                                             accum_out=cacc[:, c:c + 1])
                cp1 = persist.tile([128, 1], F32)
                nc.vector.tensor_reduce(cp1[:], cacc[:],
                                        mybir.AxisListType.X, ALU.add)
                pc = psum.tile([1, 1], F32, tag="pc")
                nc.tensor.matmul(pc[:], ones[:], cp1[:], start=True,
                                 stop=True)
                nc.vector.tensor_copy(corr[:], pc[0:1, 0:1])

            total = persist.tile([1, 1], F32)
            nc.vector.tensor_tensor(out=total[:], in0=main[:], in1=corr[:],
                                    op=ALU.subtract)
            loss = persist.tile([1, 1], F32)
            nc.vector.tensor_tensor(out=loss[:], in0=total[:], in1=rec[:],
                                    op=ALU.mult)
            nc.sync.dma_start(out=out_ap[0:1], in_=loss[0:1, 0:1])

    _split_multi_waits(nc)
    return nc


class _CachedSpmdExec:
    """Build once, execute many times via PJRT shard_map (axon path)."""

    def __init__(self, nc, n_cores):
        import jax
        from jax.sharding import Mesh, PartitionSpec
        from jax.experimental.shard_map import shard_map
        from concourse import bass2jax

        bass2jax.install_neuronx_cc_hook()
        self.n_cores = n_cores
        assert nc.dbg_addr is None

        partition_name = (nc.partition_id_tensor.name
                          if nc.partition_id_tensor else None)
        in_names, out_names, out_avals, zero_shapes = [], [], [], []
        for alloc in nc.m.functions[0].allocations:
            if not isinstance(alloc, mybir.MemoryLocationSet):
                continue
            name = alloc.memorylocations[0].name
            if alloc.kind == "ExternalInput":
                if name != partition_name:
                    in_names.append(name)
            elif alloc.kind == "ExternalOutput":
                out_names.append(name)
                shape = tuple(alloc.tensor_shape)
                dtype = mybir.dt.np(alloc.dtype)
                out_avals.append(jax.core.ShapedArray(shape, dtype))
                zero_shapes.append((shape, dtype))
        self.n_params = len(in_names)
        self.in_names = list(in_names)
        self.out_names = out_names
        self.zero_shapes = zero_shapes
        all_in_names = in_names + out_names
        if partition_name is not None:
            all_in_names.append(partition_name)

        n_outs = len(out_names)
        donate = tuple(range(self.n_params, self.n_params + n_outs))

        def _body(*args):
            operands = list(args)
            if partition_name is not None:
                operands.append(bass2jax.partition_id_tensor())
            outs = bass2jax._bass_exec_p.bind(
                *operands,
                out_avals=tuple(out_avals),
                in_names=tuple(all_in_names),
                out_names=tuple(out_names),
                lowering_input_output_aliases=(),
                sim_require_finite=True,
                sim_require_nnan=True,
                nc=nc,
            )
            return tuple(outs)

        devices = jax.devices()[:n_cores]
        assert len(devices) == n_cores
        mesh = Mesh(np.asarray(devices), ("core",))
        in_specs = (PartitionSpec("core"),) * (self.n_params + n_outs)
        out_specs = (PartitionSpec("core"),) * n_outs
        self.sharded = jax.jit(
            shard_map(_body, mesh=mesh, in_specs=in_specs,
                      out_specs=out_specs, check_rep=False),
            donate_argnums=donate, keep_unused=True,
        )

    def __call__(self, in_maps):
        n = self.n_cores
        concat_in = [
            np.concatenate([np.asarray(in_maps[c][name]) for c in range(n)],
                           axis=0)
            for name in self.in_names
        ]
        concat_zeros = [
            np.zeros((n * s[0], *s[1:]), d) for (s, d) in self.zero_shapes
        ]
        out_arrs = [np.asarray(a) for a in self.sharded(*concat_in,
                                                        *concat_zeros)]
        return [
            {name: out_arrs[i].reshape(n, *self.zero_shapes[i][0])[c]
             for i, name in enumerate(self.out_names)}
            for c in range(n)
        ]


_EXEC = None


def _get_exec():
    global _EXEC
    if _EXEC is None:
        nc = _build()
        _EXEC = _CachedSpmdExec(nc, N_CORES)
    return _EXEC


def _shard_inputs(x, y):
    x = np.ascontiguousarray(np.asarray(x, dtype=np.float32))
    y = np.asarray(y).astype(np.int32)
    R = B_FULL // N_CORES
    nb = R // 128
    iota2 = np.ascontiguousarray(
        np.broadcast_to(np.arange(C_FULL, dtype=np.int16), (128, C_FULL)))
    ident = np.eye(128, dtype=np.float32)
    in_maps = []
    for k in range(N_CORES):
        xs = x[k * R:(k + 1) * R]
        ys = np.ascontiguousarray(y[k * R:(k + 1) * R].reshape(nb, 128).T)
        in_maps.append({"x": xs, "yt": ys, "iota2": iota2, "ident": ident})
    return in_maps


def kernel(x, y):
    """Full inputs in, full output out (distributes over 8 cores inside)."""
    x = np.asarray(x)
    y = np.asarray(y)
    assert x.shape == (B_FULL, C_FULL) and y.shape == (B_FULL,)
    ex = _get_exec()
    res = ex(_shard_inputs(x, y))
    out = np.asarray(res[0]["out"]).reshape(-1)[0]
    return np.float32(out)


# revision 2
# speedup vs baseline: 1.0552x; 1.0552x over previous
"""AucLoss on 8 TRN2 NeuronCores (Bass SPMD kernel).

Reference (B=8192, C=2048, GAMA=0.3, UNK=0):
    s = sigmoid(x);  pos_i = s[i, y_i];  valid_i = (y_i != 0)
    neg_j = max_c s[j, c] over c not in {y_j, 0}
    t_j = neg_j + GAMA
    sq_sum = sum_{i valid, j} [t_j > pos_i] * (t_j - pos_i)^2
    loss = sq_sum / (p_count + 1) / (B + 1)

Distribution: data-parallel over the batch. Each core processes a
B/8-row shard of x; a split AllGather of [neg | valid*pos | valid*pos^2]
(two 6 KB collectives, the first overlapping the second half of phase 1)
makes the global vectors available everywhere; every core then computes
the identical final scalar (no further collective).

Per 128-row block (logit space; sigmoid is monotone so the masked
row-max commutes with it):
  - mask = (iota == y) * -8192   4x-mode DVE tensor_scalar (bf16 out)
  - xm   = x + mask              gpsimd(5/8 blocks) / DVE(3/8 blocks)
  - neg_logit = max over [1:C) of xm   2x tensor_scalar accum reduce
    (the [1:C) range handles the UNK column)
  - pos via the idle ACT engine: sum relu(-xm - 8000) over [1:C) is
    zero everywhere except the masked label column, where it equals
    192 - x[y]; pos = sigmoid(192 - accum) folds into one activation.
    For y == 0 rows the value is junk, but those rows are invalid and
    every downstream term multiplies them by valid == 0.
The pairwise term needs no O(B^2) work:
    sum_{i valid, j} (t_j - pos_i)^2 = B*S2 - 2*T1*S1 + T2*P
with T1 = sum t, T2 = sum t^2, S1 = sum v*pos, S2 = sum v*pos^2,
P = sum v. The margin indicator [t_j > pos_i] is enforced by a
runtime-guarded correction: if max(valid*pos) >= min(t) (checked on
device), each core computes sum relu(pos_i - t_j)^2 over all pairs via
ACT Relu + Square(accum) passes and subtracts it. In this problem's
regime (t > 1 > pos always) the guard never fires, so the correction
costs one scalar branch.

Operating range: |x| must be < ~80 (the -8192/-8000 mask offsets and
sigmoid-underflow assumptions need |x| small relative to 8192; the
reference regime is randn).

Toolchain workarounds for this container's walrus build:
  - any instruction may carry at most ONE sync wait -> extra waits are
    hoisted onto same-engine NOPs after Tile scheduling
  - custom gpsimd ucode (local_scatter etc.) does not codegen -> the
    iota and the transpose identity are tiny host-supplied constants
"""

from contextlib import ExitStack

import numpy as np

import concourse.bass as bass
import concourse.mybir as mybir
import concourse.tile as tile
from concourse.vector_clock import ScopedClock

F32 = mybir.dt.float32
I32 = mybir.dt.int32
ALU = mybir.AluOpType
ACTF = mybir.ActivationFunctionType

B_FULL, C_FULL, N_CORES, GAMA = 8192, 2048, 8, 0.3


class _PatchedTileContext(tile.TileContext):
    """TileContext whose tail drain splits sem waits one per instruction."""

    def _drain_and_barrier(self, tick_clock, wait_clock):
        nc = self.nc
        drain_inst = nc.sync.drain()
        wait_clock.add_sem_waits(
            drain_inst.ins, ScopedClock({None: tick_clock.global_clock})
        )
        si = drain_inst.ins.sync_info
        if si is not None and si.on_wait and len(si.on_wait) > 1:
            extra = list(si.on_wait[1:])
            del si.on_wait[1:]
            for w in extra:
                ni = nc.sync.nop()
                nsi = ni.ins.sync_info
                if nsi is None:
                    ni.ins.sync_info = mybir.SyncInfo(on_wait=[w], on_update=[])
                else:
                    nsi.on_wait.append(w)

        nc.all_engine_barrier()
        assert self.sems is not None
        popped = nc._tile_sem_poison_stack.pop()
        assert popped is self._sem_poison
        nc.clear_and_free_semaphores(list(self.sems.allocated().values()))
        nc.all_engine_barrier()


def _split_multi_waits(nc):
    """This walrus allows one sync wait per instruction; hoist extras onto
    same-engine NOPs inserted immediately before the owning instruction."""
    n = 0
    for f in nc.m.functions:
        for bb in f.blocks:
            out = []
            for ins in bb.instructions:
                si = ins.sync_info
                if si is not None and si.on_wait and len(si.on_wait) > 1:
                    extra = list(si.on_wait[:-1])
                    del si.on_wait[:-1]
                    for w in extra:
                        n += 1
                        out.append(mybir.InstNoOp(
                            name=f"waitnop_{n}",
                            engine=ins.engine,
                            ins=[],
                            outs=[],
                            sync_info=mybir.SyncInfo(on_wait=[w], on_update=[]),
                        ))
                out.append(ins)
            bb.instructions[:] = out
    return n


def _build(B=B_FULL, C=C_FULL, n_cores=N_CORES, gama=GAMA):
    R = B // n_cores
    nb = R // 128
    assert R % 128 == 0
    MASKVAL = -8192.0

    nc = bass.Bass("TRN2", target_bir_lowering=False, debug=False,
                   num_devices=n_cores)
    x_ap = nc.dram_tensor("x", [R, C], F32, kind="ExternalInput").ap()
    y_ap = nc.dram_tensor("yt", [128, nb], I32, kind="ExternalInput").ap()
    iota_ap = nc.dram_tensor("iota2", [128, C],
                             mybir.dt.int16, kind="ExternalInput").ap()
    ident_ap = nc.dram_tensor("ident", [128, 128], F32,
                              kind="ExternalInput").ap()
    out_ap = nc.dram_tensor("out", [1], F32, kind="ExternalOutput").ap()

    groups = [list(range(n_cores))]

    with _PatchedTileContext(nc) as tc:
        with ExitStack() as stk:
            persist = stk.enter_context(tc.tile_pool(name="persist", bufs=1))
            dram = stk.enter_context(
                tc.tile_pool(name="dram", bufs=1, space="DRAM"))
            psum = stk.enter_context(
                tc.tile_pool(name="psum", bufs=1, space="PSUM"))

            ident = persist.tile([128, 128], F32)
            nc.sync.dma_start(out=ident[:], in_=ident_ap)
            iota2 = persist.tile([128, C], mybir.dt.int16)
            nc.sync.dma_start(out=iota2[:], in_=iota_ap)
            ones = persist.tile([128, 1], F32)
            nc.vector.memset(ones[:], 1.0)
            ones2 = persist.tile([2, 1], F32)
            nc.vector.memset(ones2[:], 1.0)

            y32 = persist.tile([128, nb], I32)
            nc.sync.dma_start(out=y32[:], in_=y_ap)
            valid = persist.tile([128, nb], F32)
            nc.vector.tensor_scalar(valid[:], y32[:], 0, None, ALU.not_equal)
            yf = persist.tile([128, nb], F32)
            nc.vector.tensor_copy(yf[:], y32[:])

            negl = persist.tile([128, nb], F32)
            posm = persist.tile([128, nb], F32)
            bn8000 = persist.tile([128, 1], F32)
            nc.vector.memset(bn8000[:], -8000.0)

            # trigger the sigmoid ACT table load early so the ~2.7us
            # PSEUDO_LOAD overlaps with phase-1 DMA/compute
            warm = persist.tile([1, 1], F32)
            nc.scalar.activation(warm[:], ones[0:1, 0:1], ACTF.Sigmoid)

            # ---- phase 1: per-block masked rowmax + label extraction ----
            with tc.tile_pool(name="xp", bufs=3) as xp, \
                 tc.tile_pool(name="mp", bufs=3) as mp, \
                 tc.tile_pool(name="dp", bufs=3) as dp:
                for b in range(nb):
                    xb = xp.tile([128, C], F32, tag="x")
                    nc.sync.dma_start(out=xb[:],
                                      in_=x_ap[128 * b:128 * (b + 1), :])

                    mask = mp.tile([128, C], mybir.dt.bfloat16, tag="mask")
                    nc.vector.tensor_scalar(
                        mask[:], iota2[:], yf[:, b:b + 1], MASKVAL,
                        ALU.is_equal, ALU.mult)

                    xm = dp.tile([128, C], F32, tag="dummy")
                    # x + mask: gpsimd (default-loaded ucode lib) takes 5 of
                    # 8 blocks; DVE takes 3 to balance engine busy time
                    eng = nc.vector if b % 8 in (2, 5, 7) else nc.gpsimd
                    eng.tensor_tensor(out=xm[:], in0=xb[:], in1=mask[:],
                                      op=ALU.add)
                    jk1 = dp.tile([128, C - 1], F32, tag="jk1")
                    # masked row-max over [1:C); tensor_scalar with
                    # accum_out reduces via op1 and runs in 2x mode
                    nc.vector.tensor_scalar(
                        jk1[:], xm[:, 1:C], 0.0, None, ALU.add, ALU.max,
                        accum_out=negl[:, b:b + 1])
                    # label extraction on the (otherwise idle) ACT engine:
                    # relu(-xm - 8000) is zero everywhere except the masked
                    # label column, where it equals 192 - x[y]; the row sum
                    # is exactly that single term
                    jk2 = dp.tile([128, C - 1], F32, tag="jk2")
                    nc.scalar.activation(
                        jk2[:], xm[:, 1:C], ACTF.Relu, bias=bn8000[:],
                        scale=-1.0, accum_out=posm[:, b:b + 1])

            # ---- phase 2: per-half sigmoid/vpos + split AllGather ----
            # The first half's collective runs while phase 1 still
            # processes the second half of the blocks, hiding its latency.
            nbh = nb // 2 if (nb % 2 == 0 and nb >= 2) else nb
            halves = [(0, nbh), (nbh, nb)] if nbh != nb else [(0, nb)]
            b192 = persist.tile([128, 1], F32)
            nc.vector.memset(b192[:], 192.0)
            pos = persist.tile([128, nb], F32)
            nbg = n_cores * nb
            gall = persist.tile([128, 3, nbg], F32)
            negall = gall[:, 0, :]
            vposall = gall[:, 1, :]
            vpos2all = gall[:, 2, :]
            for lo, hi in halves:
                hb = hi - lo
                CHh = 3 * 128 * hb
                lstats = persist.tile([128, 3, hb], F32, tag=f"lst{lo}")
                neg = lstats[:, 0, :]
                vpos = lstats[:, 1, :]
                vpos2 = lstats[:, 2, :]
                nc.scalar.activation(pos[:, lo:hi], posm[:, lo:hi],
                                     ACTF.Sigmoid, bias=b192[:], scale=-1.0)
                nc.scalar.activation(neg, negl[:, lo:hi], ACTF.Sigmoid)
                nc.vector.tensor_tensor(out=vpos, in0=pos[:, lo:hi],
                                        in1=valid[:, lo:hi], op=ALU.mult)
                nc.vector.tensor_tensor(out=vpos2, in0=vpos,
                                        in1=pos[:, lo:hi], op=ALU.mult)
                chunk = dram.tile([CHh], F32, tag=f"chunk{lo}")
                nc.sync.dma_start(
                    out=chunk[:].rearrange("(s p b) -> p s b", s=3, p=128),
                    in_=lstats[:])
                ag = dram.tile([n_cores * CHh], F32, tag=f"ag{lo}")
                nc.gpsimd.collective_compute(
                    "AllGather", ALU.bypass, replica_groups=groups,
                    ins=[chunk.opt()], outs=[ag.opt()])
                # gathered slot k half [lo:hi] -> gall cols k*nb+lo .. +hi
                for s in range(3):
                    nc.sync.dma_start(
                        out=gall[:, s, :].rearrange(
                            "p (k b) -> p k b", k=n_cores)[:, :, lo:hi],
                        in_=ag[:].rearrange("(k s p b) -> s p k b",
                                            k=n_cores, s=3, p=128)[s])

            tt_ = persist.tile([128, nbg], F32)
            nc.vector.tensor_scalar(tt_[:], negall[:], float(gama), None,
                                    ALU.add)

            # per-partition stats -> ones-matmul partition sum
            stats = persist.tile([128, 5], F32)
            nc.vector.tensor_reduce(stats[:, 0:1], tt_[:],
                                    mybir.AxisListType.X, ALU.add)
            dummyg = persist.tile([128, nbg], F32)
            nc.vector.tensor_tensor(out=dummyg[:], in0=tt_[:], in1=tt_[:],
                                    op=ALU.mult)
            nc.vector.tensor_reduce(stats[:, 1:2], dummyg[:],
                                    mybir.AxisListType.X, ALU.add)
            nc.vector.tensor_reduce(stats[:, 2:3], vposall[:],
                                    mybir.AxisListType.X, ALU.add)
            nc.vector.tensor_reduce(stats[:, 3:4], vpos2all[:],
                                    mybir.AxisListType.X, ALU.add)
            # p_count: valid rows have pos = sigmoid(..) > 0 exactly
            vp01 = persist.tile([128, nbg], F32)
            nc.vector.tensor_scalar(vp01[:], vposall[:], 0.0, None,
                                    ALU.is_gt, ALU.add,
                                    accum_out=stats[:, 4:5])

            pstats = psum.tile([1, 5], F32)
            nc.tensor.matmul(pstats[:], ones[:], stats[:], start=True,
                             stop=True)
            g = persist.tile([1, 5], F32)
            nc.vector.tensor_copy(g[:], pstats[:])
            Pk = g[0:1, 4:5]

            # main = B*S2 - 2*T1*S1 + T2*P, fused:
            m2 = persist.tile([1, 1], F32)
            nc.vector.scalar_tensor_tensor(m2[:], g[0:1, 0:1], -2.0,
                                           g[0:1, 2:3], ALU.mult, ALU.mult)
            m3 = persist.tile([1, 1], F32)
            nc.vector.tensor_tensor(out=m3[:], in0=g[0:1, 1:2], in1=Pk,
                                    op=ALU.mult)
            m13 = persist.tile([1, 1], F32)
            nc.vector.scalar_tensor_tensor(m13[:], g[0:1, 3:4], float(B),
                                           m3[:], ALU.mult, ALU.add)
            main = persist.tile([1, 1], F32)
            nc.vector.tensor_tensor(out=main[:], in0=m13[:], in1=m2[:],
                                    op=ALU.add)

            corr = persist.tile([1, 1], F32)
            nc.vector.memset(corr[:], 0.0)

            # guard: max(valid*pos) >= min(t) <=> some pair has t <= pos
            mm2 = persist.tile([128, 2], F32)
            nc.vector.tensor_reduce(mm2[:, 0:1], vposall[:],
                                    mybir.AxisListType.X, ALU.max)
            negt = persist.tile([128, nbg], F32)
            nc.vector.tensor_scalar(negt[:], tt_[:], -1.0, None, ALU.mult)
            nc.vector.tensor_reduce(mm2[:, 1:2], negt[:],
                                    mybir.AxisListType.X, ALU.max)
            # cross-partition max via PE transpose, then a K=2 ones-matmul
            pmt = psum.tile([2, 128], F32, tag="pmt")
            nc.tensor.transpose(pmt[:], mm2[:], ident[:])
            gm = persist.tile([2, 1], F32)
            nc.vector.tensor_reduce(gm[:], pmt[:], mybir.AxisListType.X,
                                    ALU.max)
            pg = psum.tile([1, 1], F32, tag="pg")
            nc.tensor.matmul(pg[:], ones2[:], gm[:], start=True, stop=True)
            flag = persist.tile([1, 1], I32)
            nc.vector.tensor_scalar(flag[:], pg[:], 0.0, None, ALU.is_ge)

            # branch condition must live in a register on every engine
            tmp = nc.alloc_registers(f"corr_flag_{nc.next_id()}",
                                     mybir.ALL_ENGINES)
            nc.regs_load(tmp, flag[0:1, 0:1])
            rv = nc.snap(tmp, donate=True, min_val=0, max_val=1)
            with tc.If(rv == 1):
                # full [B, B] correction, computed redundantly per core:
                # sum over all pairs of relu(pos_i - t_j)^2
                tflat = dram.tile([B], F32)
                nc.sync.dma_start(
                    out=tflat[:].rearrange("(p b) -> p b", p=128),
                    in_=tt_[:])
                tb1 = persist.tile([1, B], F32)
                nc.sync.dma_start(out=tb1[:], in_=tflat[:].rearrange(
                    "(o n) -> o n", o=1))
                # broadcast t to all partitions via K=1 ones-matmuls
                tb = persist.tile([128, B], F32)
                onesb = persist.tile([1, 128], F32)
                nc.vector.memset(onesb[:], 1.0)
                CBC = 512
                for j in range(0, B, CBC):
                    pbc = psum.tile([128, CBC], F32, tag="pbc")
                    nc.tensor.matmul(pbc[:], onesb[:], tb1[0:1, j:j + CBC],
                                     start=True, stop=True)
                    nc.vector.tensor_copy(tb[:, j:j + CBC], pbc[:])
                cacc = persist.tile([128, nbg], F32)
                with tc.tile_pool(name="cp", bufs=1) as cp:
                    for c in range(nbg):
                        r1 = cp.tile([128, B], F32, tag="r1")
                        nc.scalar.activation(r1[:], tb[:], ACTF.Relu,
                                             bias=vposall[:, c:c + 1],
                                             scale=-1.0)
                        r2 = cp.tile([128, B], F32, tag="r2")
                        nc.scalar.activation(r2[:], r1[:], ACTF.Square,
                                             accum_out=cacc[:, c:c + 1])
                cp1 = persist.tile([128, 1], F32)
                nc.vector.tensor_reduce(cp1[:], cacc[:],
                                        mybir.AxisListType.X, ALU.add)
                pc = psum.tile([1, 1], F32, tag="pc")
                nc.tensor.matmul(pc[:], ones[:], cp1[:], start=True,
                                 stop=True)
                nc.vector.tensor_copy(corr[:], pc[0:1, 0:1])

            total = persist.tile([1, 1], F32)
            nc.vector.tensor_tensor(out=total[:], in0=main[:], in1=corr[:],
                                    op=ALU.subtract)
            den = persist.tile([1, 1], F32)
            nc.vector.tensor_scalar(den[:], Pk, 1.0, float(B) + 1.0,
                                    ALU.add, ALU.mult)
            rec = persist.tile([1, 1], F32)
            nc.vector.reciprocal(rec[:], den[:])
            loss = persist.tile([1, 1], F32)
            nc.vector.tensor_tensor(out=loss[:], in0=total[:], in1=rec[:],
                                    op=ALU.mult)
            nc.sync.dma_start(out=out_ap[0:1], in_=loss[0:1, 0:1])

    _split_multi_waits(nc)
    return nc


class _CachedSpmdExec:
    """Build once, execute many times via PJRT shard_map (axon path)."""

    def __init__(self, nc, n_cores):
        import jax
        from jax.sharding import Mesh, PartitionSpec
        from jax.experimental.shard_map import shard_map
        from concourse import bass2jax

        bass2jax.install_neuronx_cc_hook()
        self.n_cores = n_cores
        assert nc.dbg_addr is None

        partition_name = (nc.partition_id_tensor.name
                          if nc.partition_id_tensor else None)
        in_names, out_names, out_avals, zero_shapes = [], [], [], []
        for alloc in nc.m.functions[0].allocations:
            if not isinstance(alloc, mybir.MemoryLocationSet):
                continue
            name = alloc.memorylocations[0].name
            if alloc.kind == "ExternalInput":
                if name != partition_name:
                    in_names.append(name)
            elif alloc.kind == "ExternalOutput":
                out_names.append(name)
                shape = tuple(alloc.tensor_shape)
                dtype = mybir.dt.np(alloc.dtype)
                out_avals.append(jax.core.ShapedArray(shape, dtype))
                zero_shapes.append((shape, dtype))
        self.n_params = len(in_names)
        self.in_names = list(in_names)
        self.out_names = out_names
        self.zero_shapes = zero_shapes
        all_in_names = in_names + out_names
        if partition_name is not None:
            all_in_names.append(partition_name)

        n_outs = len(out_names)
        donate = tuple(range(self.n_params, self.n_params + n_outs))

        def _body(*args):
            operands = list(args)
            if partition_name is not None:
                operands.append(bass2jax.partition_id_tensor())
            outs = bass2jax._bass_exec_p.bind(
                *operands,
                out_avals=tuple(out_avals),
                in_names=tuple(all_in_names),
                out_names=tuple(out_names),
                lowering_input_output_aliases=(),
                sim_require_finite=True,
                sim_require_nnan=True,
                nc=nc,
            )
            return tuple(outs)

        devices = jax.devices()[:n_cores]
        assert len(devices) == n_cores
        mesh = Mesh(np.asarray(devices), ("core",))
        in_specs = (PartitionSpec("core"),) * (self.n_params + n_outs)
        out_specs = (PartitionSpec("core"),) * n_outs
        self.sharded = jax.jit(
            shard_map(_body, mesh=mesh, in_specs=in_specs,
                      out_specs=out_specs, check_rep=False),
            donate_argnums=donate, keep_unused=True,
        )

    def __call__(self, in_maps):
        n = self.n_cores
        concat_in = [
            np.concatenate([np.asarray(in_maps[c][name]) for c in range(n)],
                           axis=0)
            for name in self.in_names
        ]
        concat_zeros = [
            np.zeros((n * s[0], *s[1:]), d) for (s, d) in self.zero_shapes
        ]
        out_arrs = [np.asarray(a) for a in self.sharded(*concat_in,
                                                        *concat_zeros)]
        return [
            {name: out_arrs[i].reshape(n, *self.zero_shapes[i][0])[c]
             for i, name in enumerate(self.out_names)}
            for c in range(n)
        ]


_EXEC = None


def _get_exec():
    global _EXEC
    if _EXEC is None:
        nc = _build()
        _EXEC = _CachedSpmdExec(nc, N_CORES)
    return _EXEC


def _shard_inputs(x, y):
    x = np.ascontiguousarray(np.asarray(x, dtype=np.float32))
    y = np.asarray(y).astype(np.int32)
    R = B_FULL // N_CORES
    nb = R // 128
    iota2 = np.ascontiguousarray(
        np.broadcast_to(np.arange(C_FULL, dtype=np.int16), (128, C_FULL)))
    ident = np.eye(128, dtype=np.float32)
    in_maps = []
    for k in range(N_CORES):
        xs = x[k * R:(k + 1) * R]
        ys = np.ascontiguousarray(y[k * R:(k + 1) * R].reshape(nb, 128).T)
        in_maps.append({"x": xs, "yt": ys, "iota2": iota2, "ident": ident})
    return in_maps


def kernel(x, y):
    """Full inputs in, full output out (distributes over 8 cores inside)."""
    x = np.asarray(x)
    y = np.asarray(y)
    assert x.shape == (B_FULL, C_FULL) and y.shape == (B_FULL,)
    ex = _get_exec()
    res = ex(_shard_inputs(x, y))
    out = np.asarray(res[0]["out"]).reshape(-1)[0]
    return np.float32(out)
